# revision 8
# baseline (speedup 1.0000x reference)
"""DeepSeek-MoE (7 routed experts top-2 + 1 shared expert) on 8 trn2 NeuronCores.

Strategy (expert-parallel, sparse):
  - Host computes the router (sigmoid + top-k + renorm) in a JAX_PLATFORMS=cpu
    subprocess, replicating the reference's fp32 ops bit-exactly so the top-k
    selection matches.
  - Tokens are dispatched by expert id: core e (e<7) gets expert e's tokens
    (padded to capacity CA) as "batch A" plus a slice of the shared-expert
    tokens as "batch B"; core 7 gets a CA-sized shared slice as batch A.
  - Each core runs the same Bass program: swiglu(batch A, W_A) * scale_A and
    swiglu(batch B, W_B) * scale_B, bf16 matmuls with fp32 accumulation.
  - Host scatter-adds the scaled per-expert outputs into the full output.
"""

import os
import subprocess
import sys
import tempfile

import numpy as np
import ml_dtypes

import concourse.bass as bass
import concourse.mybir as mybir
import concourse.tile as tile
from concourse import bacc, bass_utils

BF16 = ml_dtypes.bfloat16
F32 = np.float32

H = 2048          # hidden size
I = 1408          # intermediate size
E = 7             # routed experts
NT = 4 * 2048     # tokens
NKT = H // 128    # 16 k-tiles over hidden
NIT = I // 128    # 11 i-tiles over intermediate

MAX_CHUNK = 896   # tokens per on-chip chunk cap (multiple of 128)

TRACE = False     # test harness can flip this to capture an NTFF profile
LAST_PERF = None  # BassKernelResults of the last run (for test harness)

_NC_CACHE = {}

_ROUTER_SRC = r"""
import sys
import numpy as np
td = sys.argv[1]
d = np.load(td + "/in.npz")
import jax
import jax.numpy as jnp
x = jnp.asarray(d["x"])
w = jnp.asarray(d["w"])
b = jnp.asarray(d["b"])
k = int(d["k"])
logits = x @ w + b
probs = jax.nn.sigmoid(logits)
scores, idx = jax.lax.top_k(probs, k)
scores = scores / jnp.sum(scores, axis=-1, keepdims=True)
np.savez(td + "/out.npz",
         idx=np.asarray(idx, dtype=np.int32),
         scores=np.asarray(scores, dtype=np.float32))
"""


def _route(x, w_router, routing_bias, top_k):
    """Top-k routing, matching the reference's fp32 CPU arithmetic.

    Returns (idx [B,S,k] int32, scores [B,S,k] f32)."""
    try:
        with tempfile.TemporaryDirectory() as td:
            np.savez(os.path.join(td, "in.npz"),
                     x=x, w=w_router, b=routing_bias, k=np.int64(top_k))
            env = dict(os.environ)
            env["JAX_PLATFORMS"] = "cpu"
            r = subprocess.run([sys.executable, "-c", _ROUTER_SRC, td],
                               env=env, capture_output=True, text=True)
            if r.returncode != 0:
                raise RuntimeError(f"router subprocess failed: {r.stderr[-2000:]}")
            d = np.load(os.path.join(td, "out.npz"))
            return d["idx"], d["scores"]
    except Exception:
        # numpy fallback (fp32, same math; top-k ties broken by lowest index)
        logits = x.astype(F32) @ w_router.astype(F32) + routing_bias.astype(F32)
        probs = 1.0 / (1.0 + np.exp(-logits))
        k = int(top_k)
        # argsort descending, stable → lowest index wins ties, like lax.top_k
        order = np.argsort(-probs, axis=-1, kind="stable")[..., :k]
        sc = np.take_along_axis(probs, order, axis=-1)
        sc = sc / sc.sum(axis=-1, keepdims=True)
        return order.astype(np.int32), sc.astype(F32)


def _subtiles(total, step):
    out = []
    s = 0
    while s < total:
        out.append((s, min(step, total - s)))
        s += step
    return out


def _balanced_chunks(T, max_chunk):
    """Split T (multiple of 128) into near-equal chunks of ≤max_chunk, each a
    multiple of 128. Avoids tiny runt chunks whose N<256 matmuls can't hide
    LDWEIGHTS."""
    tiles = T // 128
    n = -(-tiles // (max_chunk // 128))
    base, extra = divmod(tiles, n)
    out = []
    t0 = 0
    for i in range(n):
        tc = (base + (1 if i < extra else 0)) * 128
        out.append((t0, tc))
        t0 += tc
    return out


def _emit_phase(nc, pools, x_view, wg_dram, wu_dram, wd_dram, s_dram, y_dram, T):
    """swiglu over T tokens: y[t,:] = s[t] * ((silu(x@wg) * (x@wu)) @ wd)."""
    f32 = mybir.dt.float32
    bf16 = mybir.dt.bfloat16
    Silu = mybir.ActivationFunctionType.Silu
    Copy = mybir.ActivationFunctionType.Copy

    chunks = _balanced_chunks(T, MAX_CHUNK)
    max_tc = max(tc for _, tc in chunks)

    # first chunk's tokens go out on the Sync queue before anything else so
    # the PE can start ~10us in; wd / scales ride the ACT HWDGE queue.
    xt0 = pools["x"].tile([128, NKT, max_tc], bf16, tag="xt", name="xt")
    nc.sync.dma_start(out=xt0[:, :, :chunks[0][1]],
                      in_=x_view[:, :, 0:chunks[0][1]])

    s_sb = pools["const"].tile([128, T // 128], f32, tag="s", name="s_sb")
    nc.scalar.dma_start(out=s_sb, in_=s_dram)

    wd_tiles = []
    for i in range(NIT):
        wdt = pools["wd"].tile([128, H], bf16, tag=f"wd{i}", name=f"wd_sb{i}")
        nc.scalar.dma_start(out=wdt, in_=wd_dram[i])
        wd_tiles.append(wdt)

    for ci, (t0, tc) in enumerate(chunks):
        if ci == 0:
            xt = xt0
        else:
            xt = pools["x"].tile([128, NKT, max_tc], bf16, tag="xt", name="xt")
            nc.sync.dma_start(out=xt[:, :, :tc], in_=x_view[:, :, t0:t0 + tc])

        hts = []
        for i in range(NIT):
            wgt = pools["w"].tile([128, NKT, 128], bf16, tag="wg", name="wg_sb")
            wut = pools["w"].tile([128, NKT, 128], bf16, tag="wu", name="wu_sb")
            nc.sync.dma_start(out=wgt, in_=wg_dram[i])
            nc.sync.dma_start(out=wut, in_=wu_dram[i])
            ht = pools["ht"].tile([128, max_tc], bf16, tag=f"ht{i}",
                                  name=f"ht{i}")
            hts.append(ht)
            for (s0, ns) in _subtiles(tc, 512):
                pg = pools["ps1"].tile([128, 512], f32, tag="pg", name="pg")
                pu = pools["ps1"].tile([128, 512], f32, tag="pu", name="pu")
                for k in range(NKT):
                    nc.tensor.matmul(pg[:, :ns], wgt[:, k, :],
                                     xt[:, k, s0:s0 + ns],
                                     start=(k == 0), stop=(k == NKT - 1))
                for k in range(NKT):
                    nc.tensor.matmul(pu[:, :ns], wut[:, k, :],
                                     xt[:, k, s0:s0 + ns],
                                     start=(k == 0), stop=(k == NKT - 1))
                sg = pools["tmp"].tile([128, 512], f32, tag="sg", name="sg")
                nc.scalar.activation(sg[:, :ns], pg[:, :ns], Silu)
                nc.vector.tensor_mul(ht[:, s0:s0 + ns], sg[:, :ns], pu[:, :ns])

        for t128 in range(tc // 128):
            gt = t0 // 128 + t128
            ysb = pools["y"].tile([128, H], f32, tag="y", name="ysb")
            for h4 in range(H // 512):
                py = pools["ps2"].tile([128, 512], f32, tag="py", name="py")
                for i in range(NIT):
                    nc.tensor.matmul(py,
                                     hts[i][:, t128 * 128:(t128 + 1) * 128],
                                     wd_tiles[i][:, h4 * 512:(h4 + 1) * 512],
                                     start=(i == 0), stop=(i == NIT - 1))
                nc.scalar.activation(ysb[:, h4 * 512:(h4 + 1) * 512], py, Copy,
                                     scale=s_sb[:, gt:gt + 1])
            nc.scalar.dma_start(
                out=y_dram[t0 + t128 * 128:t0 + (t128 + 1) * 128, :], in_=ysb)


def _build_program(CA, CB):
    bf16 = mybir.dt.bfloat16
    f32 = mybir.dt.float32
    nc = bacc.Bacc("TRN2", target_bir_lowering=False, debug=False,
                   enable_asserts=False, num_devices=8)

    xat = nc.dram_tensor("xat", [NKT, 128, CA], bf16, kind="ExternalInput").ap()
    xbt = nc.dram_tensor("xbt", [NKT, 128, CB], bf16, kind="ExternalInput").ap()
    wga = nc.dram_tensor("wga", [NIT, 128, NKT, 128], bf16, kind="ExternalInput").ap()
    wua = nc.dram_tensor("wua", [NIT, 128, NKT, 128], bf16, kind="ExternalInput").ap()
    wda = nc.dram_tensor("wda", [NIT, 128, H], bf16, kind="ExternalInput").ap()
    wgb = nc.dram_tensor("wgb", [NIT, 128, NKT, 128], bf16, kind="ExternalInput").ap()
    wub = nc.dram_tensor("wub", [NIT, 128, NKT, 128], bf16, kind="ExternalInput").ap()
    wdb = nc.dram_tensor("wdb", [NIT, 128, H], bf16, kind="ExternalInput").ap()
    sa = nc.dram_tensor("sa", [128, CA // 128], f32, kind="ExternalInput").ap()
    sb = nc.dram_tensor("sb", [128, CB // 128], f32, kind="ExternalInput").ap()
    ya = nc.dram_tensor("ya", [CA, H], f32, kind="ExternalOutput").ap()
    yb = nc.dram_tensor("yb", [CB, H], f32, kind="ExternalOutput").ap()

    with tile.TileContext(nc) as tc:
        with tc.tile_pool(name="const", bufs=2) as p_const, \
             tc.tile_pool(name="wdp", bufs=1) as p_wd, \
             tc.tile_pool(name="xp", bufs=2) as p_x, \
             tc.tile_pool(name="wp", bufs=3) as p_w, \
             tc.tile_pool(name="htp", bufs=2) as p_ht, \
             tc.tile_pool(name="yp", bufs=2) as p_y, \
             tc.tile_pool(name="tmpp", bufs=3) as p_tmp, \
             tc.tile_pool(name="ps1", bufs=2, space="PSUM") as p_ps1, \
             tc.tile_pool(name="ps2", bufs=4, space="PSUM") as p_ps2:
            pools = {"const": p_const, "wd": p_wd, "x": p_x, "w": p_w,
                     "ht": p_ht, "y": p_y, "tmp": p_tmp,
                     "ps1": p_ps1, "ps2": p_ps2}
            _emit_phase(nc, pools, xat.rearrange("k p t -> p k t"),
                        wga, wua, wda, sa, ya, CA)
            _emit_phase(nc, pools, xbt.rearrange("k p t -> p k t"),
                        wgb, wub, wdb, sb, yb, CB)

    nc.compile()
    return nc


def _pack_gate_up(w):
    # [H, I] f32 -> [NIT, 128, NKT, 128] bf16, [i, p, k, c] = w[k*128+p, i*128+c]
    t = np.ascontiguousarray(
        w.astype(BF16).reshape(NKT, 128, NIT, 128).transpose(2, 1, 0, 3))
    return t


def _pack_down(w):
    # [I, H] f32 -> [NIT, 128, H] bf16
    return np.ascontiguousarray(w.astype(BF16).reshape(NIT, 128, H))


def _pack_tokens(x_rows, cap):
    # [n, H] f32 -> [NKT, 128, cap] bf16 (transposed, zero-padded)
    n = x_rows.shape[0]
    xt = np.zeros((H, cap), dtype=BF16)
    xt[:, :n] = x_rows.astype(BF16).T
    return np.ascontiguousarray(xt).reshape(NKT, 128, cap)


def _pack_scales(s, cap):
    # [n] f32 -> [128, cap//128] f32 where [p, j] = s[j*128+p]
    full = np.zeros(cap, dtype=F32)
    full[:s.shape[0]] = s
    return np.ascontiguousarray(full.reshape(cap // 128, 128).T)


def kernel(x, ws_gate, ws_up, ws_down, wr_gate, wr_up, wr_down,
           w_router, routing_bias, top_k):
    global LAST_PERF
    x = np.asarray(x, dtype=F32)
    ws_gate = np.asarray(ws_gate, dtype=F32)
    ws_up = np.asarray(ws_up, dtype=F32)
    ws_down = np.asarray(ws_down, dtype=F32)
    wr_gate = np.asarray(wr_gate, dtype=F32)
    wr_up = np.asarray(wr_up, dtype=F32)
    wr_down = np.asarray(wr_down, dtype=F32)
    w_router = np.asarray(w_router, dtype=F32)
    routing_bias = np.asarray(routing_bias, dtype=F32)
    k = int(top_k)

    Bv, Sv, Hv = x.shape
    nt = Bv * Sv
    x_flat = x.reshape(nt, Hv)

    idx, scores = _route(x, w_router, routing_bias, k)
    idx = idx.reshape(nt, k)
    scores = scores.reshape(nt, k).astype(F32)

    # token lists per routed expert
    tok_lists = []
    cw_lists = []
    for e in range(E):
        mask = (idx == e)
        rows = np.nonzero(mask.any(axis=1))[0]
        w = (scores * mask).sum(axis=1)[rows]
        tok_lists.append(rows)
        cw_lists.append(w.astype(F32))

    max_n = max(1, max(len(t) for t in tok_lists))
    CA = -(-max_n // 128) * 128
    rem = nt - CA
    CB = max(128, -(-rem // (8 * 128)) * 128) if rem > 0 else 128

    # shared-token slices: core 7's batch A covers [0, CA); core i's batch B
    # covers [CA + i*CB, CA + (i+1)*CB) clipped to nt
    shared_a = (0, min(CA, nt))
    shared_b = []
    for i in range(8):
        lo = min(CA + i * CB, nt)
        hi = min(CA + (i + 1) * CB, nt)
        shared_b.append((lo, hi))

    # per-core input maps
    packed_shared = (_pack_gate_up(ws_gate), _pack_gate_up(ws_up),
                     _pack_down(ws_down))
    in_maps = []
    for c in range(8):
        if c < E:
            tok = tok_lists[c]
            cw = cw_lists[c]
            wg_a = _pack_gate_up(wr_gate[c])
            wu_a = _pack_gate_up(wr_up[c])
            wd_a = _pack_down(wr_down[c])
        else:
            lo, hi = shared_a
            tok = np.arange(lo, hi)
            cw = np.ones(hi - lo, dtype=F32)
            wg_a, wu_a, wd_a = packed_shared
        lo, hi = shared_b[c]
        tok_b = np.arange(lo, hi)
        in_maps.append({
            "xat": _pack_tokens(x_flat[tok], CA),
            "xbt": _pack_tokens(x_flat[tok_b], CB),
            "wga": wg_a, "wua": wu_a, "wda": wd_a,
            "wgb": packed_shared[0], "wub": packed_shared[1],
            "wdb": packed_shared[2],
            "sa": _pack_scales(cw, CA),
            "sb": _pack_scales(np.ones(hi - lo, dtype=F32), CB),
        })

    key = (CA, CB)
    if key not in _NC_CACHE:
        _NC_CACHE[key] = _build_program(CA, CB)
    nc = _NC_CACHE[key]

    res = bass_utils.run_bass_kernel_spmd(nc, in_maps, core_ids=list(range(8)),
                                          trace=TRACE)
    LAST_PERF = res

    out = np.zeros((nt, Hv), dtype=F32)
    for c in range(8):
        ya = res.results[c]["ya"]
        yb = res.results[c]["yb"]
        if c < E:
            tok = tok_lists[c]
            out[tok] += ya[:len(tok)]
        else:
            lo, hi = shared_a
            out[lo:hi] += ya[:hi - lo]
        lo, hi = shared_b[c]
        if hi > lo:
            out[lo:hi] += yb[:hi - lo]
    return out.reshape(Bv, Sv, Hv)


# revision 12
# speedup vs baseline: 1.0103x; 1.0103x over previous
"""DeepSeek-MoE (7 routed experts top-2 + 1 shared expert) on 8 trn2 NeuronCores.

Strategy (expert-parallel, sparse):
  - Host computes the router (sigmoid + top-k + renorm) in a JAX_PLATFORMS=cpu
    subprocess, replicating the reference's fp32 ops bit-exactly so the top-k
    selection matches.
  - Tokens are dispatched by expert id: core e (e<7) gets expert e's tokens
    (padded to capacity CA) as "batch A" plus a slice of the shared-expert
    tokens as "batch B"; core 7 gets a CA-sized shared slice as batch A.
  - Each core runs the same Bass program: swiglu(batch A, W_A) * scale_A and
    swiglu(batch B, W_B) * scale_B, bf16 matmuls with fp32 accumulation.
  - Host scatter-adds the scaled per-expert outputs into the full output.
"""

import os
import subprocess
import sys
import tempfile

import numpy as np
import ml_dtypes

import concourse.bass as bass
import concourse.mybir as mybir
import concourse.tile as tile
from concourse import bacc, bass_utils

BF16 = ml_dtypes.bfloat16
F32 = np.float32

H = 2048          # hidden size
I = 1408          # intermediate size
E = 7             # routed experts
NT = 4 * 2048     # tokens
NKT = H // 128    # 16 k-tiles over hidden
NIT = I // 128    # 11 i-tiles over intermediate

MAX_CHUNK = 896   # tokens per on-chip chunk cap (multiple of 128)

TRACE = False     # test harness can flip this to capture an NTFF profile
LAST_PERF = None  # BassKernelResults of the last run (for test harness)

_NC_CACHE = {}

_ROUTER_SRC = r"""
import sys
import numpy as np
td = sys.argv[1]
d = np.load(td + "/in.npz")
import jax
import jax.numpy as jnp
x = jnp.asarray(d["x"])
w = jnp.asarray(d["w"])
b = jnp.asarray(d["b"])
k = int(d["k"])
logits = x @ w + b
probs = jax.nn.sigmoid(logits)
scores, idx = jax.lax.top_k(probs, k)
scores = scores / jnp.sum(scores, axis=-1, keepdims=True)
np.savez(td + "/out.npz",
         idx=np.asarray(idx, dtype=np.int32),
         scores=np.asarray(scores, dtype=np.float32))
"""


def _route(x, w_router, routing_bias, top_k):
    """Top-k routing, matching the reference's fp32 CPU arithmetic.

    Returns (idx [B,S,k] int32, scores [B,S,k] f32)."""
    try:
        with tempfile.TemporaryDirectory() as td:
            np.savez(os.path.join(td, "in.npz"),
                     x=x, w=w_router, b=routing_bias, k=np.int64(top_k))
            env = dict(os.environ)
            env["JAX_PLATFORMS"] = "cpu"
            r = subprocess.run([sys.executable, "-c", _ROUTER_SRC, td],
                               env=env, capture_output=True, text=True)
            if r.returncode != 0:
                raise RuntimeError(f"router subprocess failed: {r.stderr[-2000:]}")
            d = np.load(os.path.join(td, "out.npz"))
            return d["idx"], d["scores"]
    except Exception:
        # numpy fallback (fp32, same math; top-k ties broken by lowest index)
        logits = x.astype(F32) @ w_router.astype(F32) + routing_bias.astype(F32)
        probs = 1.0 / (1.0 + np.exp(-logits))
        k = int(top_k)
        # argsort descending, stable → lowest index wins ties, like lax.top_k
        order = np.argsort(-probs, axis=-1, kind="stable")[..., :k]
        sc = np.take_along_axis(probs, order, axis=-1)
        sc = sc / sc.sum(axis=-1, keepdims=True)
        return order.astype(np.int32), sc.astype(F32)


def _subtiles(total, step):
    out = []
    s = 0
    while s < total:
        out.append((s, min(step, total - s)))
        s += step
    return out


def _balanced_chunks(T, max_chunk):
    """Split T (multiple of 128) into near-equal chunks of ≤max_chunk, each a
    multiple of 128. Avoids tiny runt chunks whose N<256 matmuls can't hide
    LDWEIGHTS."""
    tiles = T // 128
    n = -(-tiles // (max_chunk // 128))
    base, extra = divmod(tiles, n)
    out = []
    t0 = 0
    for i in range(n):
        tc = (base + (1 if i < extra else 0)) * 128
        out.append((t0, tc))
        t0 += tc
    return out


def _emit_phase(nc, pools, x_view, wg_dram, wu_dram, wd_dram, s_dram, y_dram, T):
    """swiglu over T tokens: y[t,:] = s[t] * ((silu(x@wg) * (x@wu)) @ wd)."""
    f32 = mybir.dt.float32
    bf16 = mybir.dt.bfloat16
    Silu = mybir.ActivationFunctionType.Silu
    Copy = mybir.ActivationFunctionType.Copy

    chunks = _balanced_chunks(T, MAX_CHUNK)
    max_tc = max(tc for _, tc in chunks)
    s_sb = None
    wd_tiles = []

    for ci, (t0, tc) in enumerate(chunks):
        xt = pools["x"].tile([128, NKT, max_tc], bf16, tag="xt", name="xt")
        wg0 = wu0 = None
        if ci == 0:
            # critical startup path, in queue order: first i-block's weights,
            # then the first 512 token columns, then the rest of the chunk.
            wg0 = pools["w"].tile([128, NKT, 128], bf16, tag="wg", name="wg_sb")
            wu0 = pools["w"].tile([128, NKT, 128], bf16, tag="wu", name="wu_sb")
            nc.sync.dma_start(out=wg0, in_=wg_dram[0])
            nc.sync.dma_start(out=wu0, in_=wu_dram[0])
            n0 = min(512, tc)
            nc.sync.dma_start(out=xt[:, :, :n0], in_=x_view[:, :, t0:t0 + n0])
            if tc > n0:
                nc.sync.dma_start(out=xt[:, :, n0:tc],
                                  in_=x_view[:, :, t0 + n0:t0 + tc])
        else:
            nc.sync.dma_start(out=xt[:, :, :tc], in_=x_view[:, :, t0:t0 + tc])

        hts = []
        for i in range(NIT):
            if ci == 0 and i == 0:
                wgt, wut = wg0, wu0
            else:
                wgt = pools["w"].tile([128, NKT, 128], bf16, tag="wg", name="wg_sb")
                wut = pools["w"].tile([128, NKT, 128], bf16, tag="wu", name="wu_sb")
                nc.sync.dma_start(out=wgt, in_=wg_dram[i])
                nc.sync.dma_start(out=wut, in_=wu_dram[i])
            ht = pools["ht"].tile([128, max_tc], bf16, tag=f"ht{i}",
                                  name=f"ht{i}")
            hts.append(ht)
            for (s0, ns) in _subtiles(tc, 512):
                pg = pools["ps1"].tile([128, 512], f32, tag="pg", name="pg",
                                       bufs=3)
                pu = pools["ps1"].tile([128, 512], f32, tag="pu", name="pu",
                                       bufs=2)
                for k in range(NKT):
                    nc.tensor.matmul(pg[:, :ns], wgt[:, k, :],
                                     xt[:, k, s0:s0 + ns],
                                     start=(k == 0), stop=(k == NKT - 1))
                for k in range(NKT):
                    nc.tensor.matmul(pu[:, :ns], wut[:, k, :],
                                     xt[:, k, s0:s0 + ns],
                                     start=(k == 0), stop=(k == NKT - 1))
                sg = pools["tmp"].tile([128, 512], f32, tag="sg", name="sg")
                nc.scalar.activation(sg[:, :ns], pg[:, :ns], Silu)
                nc.vector.tensor_mul(ht[:, s0:s0 + ns], sg[:, :ns], pu[:, :ns])

        if ci == 0:
            # wd / scales are first needed by stage 2 of chunk 0 — keep them
            # behind the stage-1 weight streams in the queue.
            s_sb = pools["const"].tile([128, T // 128], f32, tag="s", name="s_sb")
            nc.scalar.dma_start(out=s_sb, in_=s_dram)
            for i in range(NIT):
                wdt = pools["wd"].tile([128, H], bf16, tag=f"wd{i}",
                                       name=f"wd_sb{i}")
                nc.scalar.dma_start(out=wdt, in_=wd_dram[i])
                wd_tiles.append(wdt)

        for t128 in range(tc // 128):
            gt = t0 // 128 + t128
            ysb = pools["y"].tile([128, H], f32, tag="y", name="ysb")
            for h4 in range(H // 512):
                py = pools["ps2"].tile([128, 512], f32, tag="py", name="py")
                for i in range(NIT):
                    nc.tensor.matmul(py,
                                     hts[i][:, t128 * 128:(t128 + 1) * 128],
                                     wd_tiles[i][:, h4 * 512:(h4 + 1) * 512],
                                     start=(i == 0), stop=(i == NIT - 1))
                nc.scalar.activation(ysb[:, h4 * 512:(h4 + 1) * 512], py, Copy,
                                     scale=s_sb[:, gt:gt + 1])
            nc.scalar.dma_start(
                out=y_dram[t0 + t128 * 128:t0 + (t128 + 1) * 128, :], in_=ysb)


def _build_program(CA, CB):
    bf16 = mybir.dt.bfloat16
    f32 = mybir.dt.float32
    nc = bacc.Bacc("TRN2", target_bir_lowering=False, debug=False,
                   enable_asserts=False, num_devices=8)

    xat = nc.dram_tensor("xat", [NKT, 128, CA], bf16, kind="ExternalInput").ap()
    xbt = nc.dram_tensor("xbt", [NKT, 128, CB], bf16, kind="ExternalInput").ap()
    wga = nc.dram_tensor("wga", [NIT, 128, NKT, 128], bf16, kind="ExternalInput").ap()
    wua = nc.dram_tensor("wua", [NIT, 128, NKT, 128], bf16, kind="ExternalInput").ap()
    wda = nc.dram_tensor("wda", [NIT, 128, H], bf16, kind="ExternalInput").ap()
    wgb = nc.dram_tensor("wgb", [NIT, 128, NKT, 128], bf16, kind="ExternalInput").ap()
    wub = nc.dram_tensor("wub", [NIT, 128, NKT, 128], bf16, kind="ExternalInput").ap()
    wdb = nc.dram_tensor("wdb", [NIT, 128, H], bf16, kind="ExternalInput").ap()
    sa = nc.dram_tensor("sa", [128, CA // 128], f32, kind="ExternalInput").ap()
    sb = nc.dram_tensor("sb", [128, CB // 128], f32, kind="ExternalInput").ap()
    ya = nc.dram_tensor("ya", [CA, H], f32, kind="ExternalOutput").ap()
    yb = nc.dram_tensor("yb", [CB, H], f32, kind="ExternalOutput").ap()

    with tile.TileContext(nc) as tc:
        with tc.tile_pool(name="const", bufs=2) as p_const, \
             tc.tile_pool(name="wdp", bufs=1) as p_wd, \
             tc.tile_pool(name="xp", bufs=2) as p_x, \
             tc.tile_pool(name="wp", bufs=3) as p_w, \
             tc.tile_pool(name="htp", bufs=2) as p_ht, \
             tc.tile_pool(name="yp", bufs=2) as p_y, \
             tc.tile_pool(name="tmpp", bufs=3) as p_tmp, \
             tc.tile_pool(name="ps1", bufs=2, space="PSUM") as p_ps1, \
             tc.tile_pool(name="ps2", bufs=3, space="PSUM") as p_ps2:
            pools = {"const": p_const, "wd": p_wd, "x": p_x, "w": p_w,
                     "ht": p_ht, "y": p_y, "tmp": p_tmp,
                     "ps1": p_ps1, "ps2": p_ps2}
            _emit_phase(nc, pools, xat.rearrange("k p t -> p k t"),
                        wga, wua, wda, sa, ya, CA)
            _emit_phase(nc, pools, xbt.rearrange("k p t -> p k t"),
                        wgb, wub, wdb, sb, yb, CB)

    nc.compile()
    return nc


def _pack_gate_up(w):
    # [H, I] f32 -> [NIT, 128, NKT, 128] bf16, [i, p, k, c] = w[k*128+p, i*128+c]
    t = np.ascontiguousarray(
        w.astype(BF16).reshape(NKT, 128, NIT, 128).transpose(2, 1, 0, 3))
    return t


def _pack_down(w):
    # [I, H] f32 -> [NIT, 128, H] bf16
    return np.ascontiguousarray(w.astype(BF16).reshape(NIT, 128, H))


def _pack_tokens(x_rows, cap):
    # [n, H] f32 -> [NKT, 128, cap] bf16 (transposed, zero-padded)
    n = x_rows.shape[0]
    xt = np.zeros((H, cap), dtype=BF16)
    xt[:, :n] = x_rows.astype(BF16).T
    return np.ascontiguousarray(xt).reshape(NKT, 128, cap)


def _pack_scales(s, cap):
    # [n] f32 -> [128, cap//128] f32 where [p, j] = s[j*128+p]
    full = np.zeros(cap, dtype=F32)
    full[:s.shape[0]] = s
    return np.ascontiguousarray(full.reshape(cap // 128, 128).T)


def kernel(x, ws_gate, ws_up, ws_down, wr_gate, wr_up, wr_down,
           w_router, routing_bias, top_k):
    global LAST_PERF
    x = np.asarray(x, dtype=F32)
    ws_gate = np.asarray(ws_gate, dtype=F32)
    ws_up = np.asarray(ws_up, dtype=F32)
    ws_down = np.asarray(ws_down, dtype=F32)
    wr_gate = np.asarray(wr_gate, dtype=F32)
    wr_up = np.asarray(wr_up, dtype=F32)
    wr_down = np.asarray(wr_down, dtype=F32)
    w_router = np.asarray(w_router, dtype=F32)
    routing_bias = np.asarray(routing_bias, dtype=F32)
    k = int(top_k)

    Bv, Sv, Hv = x.shape
    nt = Bv * Sv
    x_flat = x.reshape(nt, Hv)

    idx, scores = _route(x, w_router, routing_bias, k)
    idx = idx.reshape(nt, k)
    scores = scores.reshape(nt, k).astype(F32)

    # token lists per routed expert
    tok_lists = []
    cw_lists = []
    for e in range(E):
        mask = (idx == e)
        rows = np.nonzero(mask.any(axis=1))[0]
        w = (scores * mask).sum(axis=1)[rows]
        tok_lists.append(rows)
        cw_lists.append(w.astype(F32))

    max_n = max(1, max(len(t) for t in tok_lists))
    CA = -(-max_n // 128) * 128
    rem = nt - CA
    CB = max(128, -(-rem // (8 * 128)) * 128) if rem > 0 else 128

    # shared-token slices: core 7's batch A covers [0, CA); core i's batch B
    # covers [CA + i*CB, CA + (i+1)*CB) clipped to nt
    shared_a = (0, min(CA, nt))
    shared_b = []
    for i in range(8):
        lo = min(CA + i * CB, nt)
        hi = min(CA + (i + 1) * CB, nt)
        shared_b.append((lo, hi))

    # per-core input maps
    packed_shared = (_pack_gate_up(ws_gate), _pack_gate_up(ws_up),
                     _pack_down(ws_down))
    in_maps = []
    for c in range(8):
        if c < E:
            tok = tok_lists[c]
            cw = cw_lists[c]
            wg_a = _pack_gate_up(wr_gate[c])
            wu_a = _pack_gate_up(wr_up[c])
            wd_a = _pack_down(wr_down[c])
        else:
            lo, hi = shared_a
            tok = np.arange(lo, hi)
            cw = np.ones(hi - lo, dtype=F32)
            wg_a, wu_a, wd_a = packed_shared
        lo, hi = shared_b[c]
        tok_b = np.arange(lo, hi)
        in_maps.append({
            "xat": _pack_tokens(x_flat[tok], CA),
            "xbt": _pack_tokens(x_flat[tok_b], CB),
            "wga": wg_a, "wua": wu_a, "wda": wd_a,
            "wgb": packed_shared[0], "wub": packed_shared[1],
            "wdb": packed_shared[2],
            "sa": _pack_scales(cw, CA),
            "sb": _pack_scales(np.ones(hi - lo, dtype=F32), CB),
        })

    key = (CA, CB)
    if key not in _NC_CACHE:
        _NC_CACHE[key] = _build_program(CA, CB)
    nc = _NC_CACHE[key]

    res = bass_utils.run_bass_kernel_spmd(nc, in_maps, core_ids=list(range(8)),
                                          trace=TRACE)
    LAST_PERF = res

    out = np.zeros((nt, Hv), dtype=F32)
    for c in range(8):
        ya = res.results[c]["ya"]
        yb = res.results[c]["yb"]
        if c < E:
            tok = tok_lists[c]
            out[tok] += ya[:len(tok)]
        else:
            lo, hi = shared_a
            out[lo:hi] += ya[:hi - lo]
        lo, hi = shared_b[c]
        if hi > lo:
            out[lo:hi] += yb[:hi - lo]
    return out.reshape(Bv, Sv, Hv)


# revision 13
# speedup vs baseline: 1.0285x; 1.0181x over previous
"""DeepSeek-MoE (7 routed experts top-2 + 1 shared expert) on 8 trn2 NeuronCores.

Strategy (expert-parallel, sparse):
  - Host computes the router (sigmoid + top-k + renorm) in a JAX_PLATFORMS=cpu
    subprocess, replicating the reference's fp32 ops bit-exactly so the top-k
    selection matches.
  - Tokens are dispatched by expert id: core e (e<7) gets expert e's tokens
    (padded to capacity CA) as "batch A" plus a slice of the shared-expert
    tokens as "batch B"; core 7 gets a CA-sized shared slice as batch A.
  - Each core runs the same Bass program: swiglu(batch A, W_A) * scale_A and
    swiglu(batch B, W_B) * scale_B, bf16 matmuls with fp32 accumulation.
  - Host scatter-adds the scaled per-expert outputs into the full output.
"""

import os
import subprocess
import sys
import tempfile

import numpy as np
import ml_dtypes

import concourse.bass as bass
import concourse.mybir as mybir
import concourse.tile as tile
from concourse import bacc, bass_utils

BF16 = ml_dtypes.bfloat16
F32 = np.float32

H = 2048          # hidden size
I = 1408          # intermediate size
E = 7             # routed experts
NT = 4 * 2048     # tokens
NKT = H // 128    # 16 k-tiles over hidden
NIT = I // 128    # 11 i-tiles over intermediate

MAX_CHUNK = 896   # tokens per on-chip chunk cap (multiple of 128)

TRACE = False     # test harness can flip this to capture an NTFF profile
LAST_PERF = None  # BassKernelResults of the last run (for test harness)

_NC_CACHE = {}

_ROUTER_SRC = r"""
import sys
import numpy as np
td = sys.argv[1]
d = np.load(td + "/in.npz")
import jax
import jax.numpy as jnp
x = jnp.asarray(d["x"])
w = jnp.asarray(d["w"])
b = jnp.asarray(d["b"])
k = int(d["k"])
logits = x @ w + b
probs = jax.nn.sigmoid(logits)
scores, idx = jax.lax.top_k(probs, k)
scores = scores / jnp.sum(scores, axis=-1, keepdims=True)
np.savez(td + "/out.npz",
         idx=np.asarray(idx, dtype=np.int32),
         scores=np.asarray(scores, dtype=np.float32))
"""


def _route(x, w_router, routing_bias, top_k):
    """Top-k routing, matching the reference's fp32 CPU arithmetic.

    Returns (idx [B,S,k] int32, scores [B,S,k] f32)."""
    try:
        with tempfile.TemporaryDirectory() as td:
            np.savez(os.path.join(td, "in.npz"),
                     x=x, w=w_router, b=routing_bias, k=np.int64(top_k))
            env = dict(os.environ)
            env["JAX_PLATFORMS"] = "cpu"
            r = subprocess.run([sys.executable, "-c", _ROUTER_SRC, td],
                               env=env, capture_output=True, text=True)
            if r.returncode != 0:
                raise RuntimeError(f"router subprocess failed: {r.stderr[-2000:]}")
            d = np.load(os.path.join(td, "out.npz"))
            return d["idx"], d["scores"]
    except Exception:
        # numpy fallback (fp32, same math; top-k ties broken by lowest index)
        logits = x.astype(F32) @ w_router.astype(F32) + routing_bias.astype(F32)
        probs = 1.0 / (1.0 + np.exp(-logits))
        k = int(top_k)
        # argsort descending, stable → lowest index wins ties, like lax.top_k
        order = np.argsort(-probs, axis=-1, kind="stable")[..., :k]
        sc = np.take_along_axis(probs, order, axis=-1)
        sc = sc / sc.sum(axis=-1, keepdims=True)
        return order.astype(np.int32), sc.astype(F32)


def _subtiles(total, step):
    out = []
    s = 0
    while s < total:
        out.append((s, min(step, total - s)))
        s += step
    return out


def _balanced_chunks(T, max_chunk):
    """Split T (multiple of 128) into near-equal chunks of ≤max_chunk, each a
    multiple of 128. Avoids tiny runt chunks whose N<256 matmuls can't hide
    LDWEIGHTS."""
    tiles = T // 128
    n = -(-tiles // (max_chunk // 128))
    base, extra = divmod(tiles, n)
    out = []
    t0 = 0
    for i in range(n):
        tc = (base + (1 if i < extra else 0)) * 128
        out.append((t0, tc))
        t0 += tc
    return out


def _emit_phase(nc, pools, x_view, wg_dram, wu_dram, wd_dram, s_dram, y_dram, T):
    """swiglu over T tokens: y[t,:] = s[t] * ((silu(x@wg) * (x@wu)) @ wd)."""
    f32 = mybir.dt.float32
    bf16 = mybir.dt.bfloat16
    Silu = mybir.ActivationFunctionType.Silu
    Copy = mybir.ActivationFunctionType.Copy

    chunks = _balanced_chunks(T, MAX_CHUNK)
    max_tc = max(tc for _, tc in chunks)
    s_sb = None
    wd_tiles = []

    for ci, (t0, tc) in enumerate(chunks):
        xt = pools["x"].tile([128, NKT, max_tc], bf16, tag="xt", name="xt")
        wg0 = wu0 = None
        if ci == 0:
            # critical startup path, in queue order: first i-block's weights,
            # then the first 512 token columns, then the rest of the chunk.
            wg0 = pools["w"].tile([128, NKT, 128], bf16, tag="wg", name="wg_sb")
            wu0 = pools["w"].tile([128, NKT, 128], bf16, tag="wu", name="wu_sb")
            nc.sync.dma_start(out=wg0, in_=wg_dram[0])
            nc.sync.dma_start(out=wu0, in_=wu_dram[0])
            n0 = min(512, tc)
            nc.sync.dma_start(out=xt[:, :, :n0], in_=x_view[:, :, t0:t0 + n0])
            if tc > n0:
                nc.sync.dma_start(out=xt[:, :, n0:tc],
                                  in_=x_view[:, :, t0 + n0:t0 + tc])
        else:
            nc.sync.dma_start(out=xt[:, :, :tc], in_=x_view[:, :, t0:t0 + tc])

        hts = []
        for i in range(NIT):
            if ci == 0 and i == 0:
                wgt, wut = wg0, wu0
            else:
                wgt = pools["w"].tile([128, NKT, 128], bf16, tag="wg", name="wg_sb")
                wut = pools["w"].tile([128, NKT, 128], bf16, tag="wu", name="wu_sb")
                nc.sync.dma_start(out=wgt, in_=wg_dram[i])
                nc.sync.dma_start(out=wut, in_=wu_dram[i])
            ht = pools["ht"].tile([128, max_tc], bf16, tag=f"ht{i}",
                                  name=f"ht{i}")
            hts.append(ht)
            for (s0, ns) in _subtiles(tc, 512):
                pg = pools["ps1"].tile([128, 512], f32, tag="pg", name="pg",
                                       bufs=3)
                pu = pools["ps1"].tile([128, 512], f32, tag="pu", name="pu",
                                       bufs=2)
                for k in range(NKT):
                    nc.tensor.matmul(pg[:, :ns], wgt[:, k, :],
                                     xt[:, k, s0:s0 + ns],
                                     start=(k == 0), stop=(k == NKT - 1))
                for k in range(NKT):
                    nc.tensor.matmul(pu[:, :ns], wut[:, k, :],
                                     xt[:, k, s0:s0 + ns],
                                     start=(k == 0), stop=(k == NKT - 1))
                sg = pools["tmp"].tile([128, 512], f32, tag="sg", name="sg")
                nc.scalar.activation(sg[:, :ns], pg[:, :ns], Silu)
                nc.vector.tensor_mul(ht[:, s0:s0 + ns], sg[:, :ns], pu[:, :ns])

        if ci == 0:
            # wd / scales are first needed by stage 2 of chunk 0 — same queue
            # as the stage-1 streams, behind chunk 0's weights (always-ready
            # DMAs keep program order within one engine queue, so these can't
            # be hoisted ahead of the startup-critical bytes).
            s_sb = pools["const"].tile([128, T // 128], f32, tag="s", name="s_sb")
            nc.sync.dma_start(out=s_sb, in_=s_dram)
            for i in range(NIT):
                wdt = pools["wd"].tile([128, H], bf16, tag=f"wd{i}",
                                       name=f"wd_sb{i}")
                nc.sync.dma_start(out=wdt, in_=wd_dram[i])
                wd_tiles.append(wdt)

        for t128 in range(tc // 128):
            gt = t0 // 128 + t128
            ysb = pools["y"].tile([128, H], f32, tag="y", name="ysb")
            for h4 in range(H // 512):
                py = pools["ps2"].tile([128, 512], f32, tag="py", name="py")
                for i in range(NIT):
                    nc.tensor.matmul(py,
                                     hts[i][:, t128 * 128:(t128 + 1) * 128],
                                     wd_tiles[i][:, h4 * 512:(h4 + 1) * 512],
                                     start=(i == 0), stop=(i == NIT - 1))
                nc.scalar.activation(ysb[:, h4 * 512:(h4 + 1) * 512], py, Copy,
                                     scale=s_sb[:, gt:gt + 1])
            nc.scalar.dma_start(
                out=y_dram[t0 + t128 * 128:t0 + (t128 + 1) * 128, :], in_=ysb)


def _build_program(CA, CB):
    bf16 = mybir.dt.bfloat16
    f32 = mybir.dt.float32
    nc = bacc.Bacc("TRN2", target_bir_lowering=False, debug=False,
                   enable_asserts=False, num_devices=8)

    xat = nc.dram_tensor("xat", [NKT, 128, CA], bf16, kind="ExternalInput").ap()
    xbt = nc.dram_tensor("xbt", [NKT, 128, CB], bf16, kind="ExternalInput").ap()
    wga = nc.dram_tensor("wga", [NIT, 128, NKT, 128], bf16, kind="ExternalInput").ap()
    wua = nc.dram_tensor("wua", [NIT, 128, NKT, 128], bf16, kind="ExternalInput").ap()
    wda = nc.dram_tensor("wda", [NIT, 128, H], bf16, kind="ExternalInput").ap()
    wgb = nc.dram_tensor("wgb", [NIT, 128, NKT, 128], bf16, kind="ExternalInput").ap()
    wub = nc.dram_tensor("wub", [NIT, 128, NKT, 128], bf16, kind="ExternalInput").ap()
    wdb = nc.dram_tensor("wdb", [NIT, 128, H], bf16, kind="ExternalInput").ap()
    sa = nc.dram_tensor("sa", [128, CA // 128], f32, kind="ExternalInput").ap()
    sb = nc.dram_tensor("sb", [128, CB // 128], f32, kind="ExternalInput").ap()
    ya = nc.dram_tensor("ya", [CA, H], f32, kind="ExternalOutput").ap()
    yb = nc.dram_tensor("yb", [CB, H], f32, kind="ExternalOutput").ap()

    with tile.TileContext(nc) as tc:
        with tc.tile_pool(name="const", bufs=2) as p_const, \
             tc.tile_pool(name="wdp", bufs=1) as p_wd, \
             tc.tile_pool(name="xp", bufs=2) as p_x, \
             tc.tile_pool(name="wp", bufs=3) as p_w, \
             tc.tile_pool(name="htp", bufs=2) as p_ht, \
             tc.tile_pool(name="yp", bufs=2) as p_y, \
             tc.tile_pool(name="tmpp", bufs=3) as p_tmp, \
             tc.tile_pool(name="ps1", bufs=2, space="PSUM") as p_ps1, \
             tc.tile_pool(name="ps2", bufs=3, space="PSUM") as p_ps2:
            pools = {"const": p_const, "wd": p_wd, "x": p_x, "w": p_w,
                     "ht": p_ht, "y": p_y, "tmp": p_tmp,
                     "ps1": p_ps1, "ps2": p_ps2}
            _emit_phase(nc, pools, xat.rearrange("k p t -> p k t"),
                        wga, wua, wda, sa, ya, CA)
            _emit_phase(nc, pools, xbt.rearrange("k p t -> p k t"),
                        wgb, wub, wdb, sb, yb, CB)

    nc.compile()
    return nc


def _pack_gate_up(w):
    # [H, I] f32 -> [NIT, 128, NKT, 128] bf16, [i, p, k, c] = w[k*128+p, i*128+c]
    t = np.ascontiguousarray(
        w.astype(BF16).reshape(NKT, 128, NIT, 128).transpose(2, 1, 0, 3))
    return t


def _pack_down(w):
    # [I, H] f32 -> [NIT, 128, H] bf16
    return np.ascontiguousarray(w.astype(BF16).reshape(NIT, 128, H))


def _pack_tokens(x_rows, cap):
    # [n, H] f32 -> [NKT, 128, cap] bf16 (transposed, zero-padded)
    n = x_rows.shape[0]
    xt = np.zeros((H, cap), dtype=BF16)
    xt[:, :n] = x_rows.astype(BF16).T
    return np.ascontiguousarray(xt).reshape(NKT, 128, cap)


def _pack_scales(s, cap):
    # [n] f32 -> [128, cap//128] f32 where [p, j] = s[j*128+p]
    full = np.zeros(cap, dtype=F32)
    full[:s.shape[0]] = s
    return np.ascontiguousarray(full.reshape(cap // 128, 128).T)


def kernel(x, ws_gate, ws_up, ws_down, wr_gate, wr_up, wr_down,
           w_router, routing_bias, top_k):
    global LAST_PERF
    x = np.asarray(x, dtype=F32)
    ws_gate = np.asarray(ws_gate, dtype=F32)
    ws_up = np.asarray(ws_up, dtype=F32)
    ws_down = np.asarray(ws_down, dtype=F32)
    wr_gate = np.asarray(wr_gate, dtype=F32)
    wr_up = np.asarray(wr_up, dtype=F32)
    wr_down = np.asarray(wr_down, dtype=F32)
    w_router = np.asarray(w_router, dtype=F32)
    routing_bias = np.asarray(routing_bias, dtype=F32)
    k = int(top_k)

    Bv, Sv, Hv = x.shape
    nt = Bv * Sv
    x_flat = x.reshape(nt, Hv)

    idx, scores = _route(x, w_router, routing_bias, k)
    idx = idx.reshape(nt, k)
    scores = scores.reshape(nt, k).astype(F32)

    # token lists per routed expert
    tok_lists = []
    cw_lists = []
    for e in range(E):
        mask = (idx == e)
        rows = np.nonzero(mask.any(axis=1))[0]
        w = (scores * mask).sum(axis=1)[rows]
        tok_lists.append(rows)
        cw_lists.append(w.astype(F32))

    max_n = max(1, max(len(t) for t in tok_lists))
    CA = -(-max_n // 128) * 128
    rem = nt - CA
    CB = max(128, -(-rem // (8 * 128)) * 128) if rem > 0 else 128

    # shared-token slices: core 7's batch A covers [0, CA); core i's batch B
    # covers [CA + i*CB, CA + (i+1)*CB) clipped to nt
    shared_a = (0, min(CA, nt))
    shared_b = []
    for i in range(8):
        lo = min(CA + i * CB, nt)
        hi = min(CA + (i + 1) * CB, nt)
        shared_b.append((lo, hi))

    # per-core input maps
    packed_shared = (_pack_gate_up(ws_gate), _pack_gate_up(ws_up),
                     _pack_down(ws_down))
    in_maps = []
    for c in range(8):
        if c < E:
            tok = tok_lists[c]
            cw = cw_lists[c]
            wg_a = _pack_gate_up(wr_gate[c])
            wu_a = _pack_gate_up(wr_up[c])
            wd_a = _pack_down(wr_down[c])
        else:
            lo, hi = shared_a
            tok = np.arange(lo, hi)
            cw = np.ones(hi - lo, dtype=F32)
            wg_a, wu_a, wd_a = packed_shared
        lo, hi = shared_b[c]
        tok_b = np.arange(lo, hi)
        in_maps.append({
            "xat": _pack_tokens(x_flat[tok], CA),
            "xbt": _pack_tokens(x_flat[tok_b], CB),
            "wga": wg_a, "wua": wu_a, "wda": wd_a,
            "wgb": packed_shared[0], "wub": packed_shared[1],
            "wdb": packed_shared[2],
            "sa": _pack_scales(cw, CA),
            "sb": _pack_scales(np.ones(hi - lo, dtype=F32), CB),
        })

    key = (CA, CB)
    if key not in _NC_CACHE:
        _NC_CACHE[key] = _build_program(CA, CB)
    nc = _NC_CACHE[key]

    res = bass_utils.run_bass_kernel_spmd(nc, in_maps, core_ids=list(range(8)),
                                          trace=TRACE)
    LAST_PERF = res

    out = np.zeros((nt, Hv), dtype=F32)
    for c in range(8):
        ya = res.results[c]["ya"]
        yb = res.results[c]["yb"]
        if c < E:
            tok = tok_lists[c]
            out[tok] += ya[:len(tok)]
        else:
            lo, hi = shared_a
            out[lo:hi] += ya[:hi - lo]
        lo, hi = shared_b[c]
        if hi > lo:
            out[lo:hi] += yb[:hi - lo]
    return out.reshape(Bv, Sv, Hv)


# revision 14
# speedup vs baseline: 1.0316x; 1.0030x over previous
"""DeepSeek-MoE (7 routed experts top-2 + 1 shared expert) on 8 trn2 NeuronCores.

Strategy (expert-parallel, sparse):
  - Host computes the router (sigmoid + top-k + renorm) in a JAX_PLATFORMS=cpu
    subprocess, replicating the reference's fp32 ops bit-exactly so the top-k
    selection matches.
  - Tokens are dispatched by expert id: core e (e<7) gets expert e's tokens
    (padded to capacity CA) as "batch A" plus a slice of the shared-expert
    tokens as "batch B"; core 7 gets a CA-sized shared slice as batch A.
  - Each core runs the same Bass program: swiglu(batch A, W_A) * scale_A and
    swiglu(batch B, W_B) * scale_B, bf16 matmuls with fp32 accumulation.
  - Host scatter-adds the scaled per-expert outputs into the full output.
"""

import os
import subprocess
import sys
import tempfile

import numpy as np
import ml_dtypes

import concourse.bass as bass
import concourse.mybir as mybir
import concourse.tile as tile
from concourse import bacc, bass_utils

BF16 = ml_dtypes.bfloat16
F32 = np.float32

H = 2048          # hidden size
I = 1408          # intermediate size
E = 7             # routed experts
NT = 4 * 2048     # tokens
NKT = H // 128    # 16 k-tiles over hidden
NIT = I // 128    # 11 i-tiles over intermediate

MAX_CHUNK = 896   # tokens per on-chip chunk cap (multiple of 128)

TRACE = False     # test harness can flip this to capture an NTFF profile
LAST_PERF = None  # BassKernelResults of the last run (for test harness)

_NC_CACHE = {}

_ROUTER_SRC = r"""
import sys
import numpy as np
td = sys.argv[1]
d = np.load(td + "/in.npz")
import jax
import jax.numpy as jnp
x = jnp.asarray(d["x"])
w = jnp.asarray(d["w"])
b = jnp.asarray(d["b"])
k = int(d["k"])
logits = x @ w + b
probs = jax.nn.sigmoid(logits)
scores, idx = jax.lax.top_k(probs, k)
scores = scores / jnp.sum(scores, axis=-1, keepdims=True)
np.savez(td + "/out.npz",
         idx=np.asarray(idx, dtype=np.int32),
         scores=np.asarray(scores, dtype=np.float32))
"""


def _route(x, w_router, routing_bias, top_k):
    """Top-k routing, matching the reference's fp32 CPU arithmetic.

    Returns (idx [B,S,k] int32, scores [B,S,k] f32)."""
    try:
        with tempfile.TemporaryDirectory() as td:
            np.savez(os.path.join(td, "in.npz"),
                     x=x, w=w_router, b=routing_bias, k=np.int64(top_k))
            env = dict(os.environ)
            env["JAX_PLATFORMS"] = "cpu"
            r = subprocess.run([sys.executable, "-c", _ROUTER_SRC, td],
                               env=env, capture_output=True, text=True)
            if r.returncode != 0:
                raise RuntimeError(f"router subprocess failed: {r.stderr[-2000:]}")
            d = np.load(os.path.join(td, "out.npz"))
            return d["idx"], d["scores"]
    except Exception:
        # numpy fallback (fp32, same math; top-k ties broken by lowest index)
        logits = x.astype(F32) @ w_router.astype(F32) + routing_bias.astype(F32)
        probs = 1.0 / (1.0 + np.exp(-logits))
        k = int(top_k)
        # argsort descending, stable → lowest index wins ties, like lax.top_k
        order = np.argsort(-probs, axis=-1, kind="stable")[..., :k]
        sc = np.take_along_axis(probs, order, axis=-1)
        sc = sc / sc.sum(axis=-1, keepdims=True)
        return order.astype(np.int32), sc.astype(F32)


def _subtiles(total, step):
    out = []
    s = 0
    while s < total:
        out.append((s, min(step, total - s)))
        s += step
    return out


def _balanced_chunks(T, max_chunk):
    """Split T (multiple of 128) into near-equal chunks of ≤max_chunk, each a
    multiple of 128. Avoids tiny runt chunks whose N<256 matmuls can't hide
    LDWEIGHTS."""
    tiles = T // 128
    n = -(-tiles // (max_chunk // 128))
    base, extra = divmod(tiles, n)
    out = []
    t0 = 0
    for i in range(n):
        tc = (base + (1 if i < extra else 0)) * 128
        out.append((t0, tc))
        t0 += tc
    return out


def _emit_phase(nc, pools, x_view, wgu_dram, wd_dram, s_dram, y_dram, T):
    """swiglu over T tokens: y[t,:] = s[t] * ((silu(x@wg) * (x@wu)) @ wd)."""
    f32 = mybir.dt.float32
    bf16 = mybir.dt.bfloat16
    Silu = mybir.ActivationFunctionType.Silu
    Copy = mybir.ActivationFunctionType.Copy

    chunks = _balanced_chunks(T, MAX_CHUNK)
    max_tc = max(tc for _, tc in chunks)
    s_sb = None
    wd_tiles = []

    for ci, (t0, tc) in enumerate(chunks):
        xt = pools["x"].tile([128, NKT, max_tc], bf16, tag="xt", name="xt")
        wgu0 = None
        if ci == 0:
            # critical startup path, in queue order: first i-block's weights,
            # then the first 512 token columns, then the rest of the chunk.
            wgu0 = pools["w"].tile([128, NKT, 256], bf16, tag="wgu", name="wgu_sb")
            nc.sync.dma_start(out=wgu0, in_=wgu_dram[0])
            n0 = min(512, tc)
            nc.sync.dma_start(out=xt[:, :, :n0], in_=x_view[:, :, t0:t0 + n0])
            if tc > n0:
                nc.sync.dma_start(out=xt[:, :, n0:tc],
                                  in_=x_view[:, :, t0 + n0:t0 + tc])
        else:
            nc.sync.dma_start(out=xt[:, :, :tc], in_=x_view[:, :, t0:t0 + tc])

        hts = []
        for i in range(NIT):
            if ci == 0 and i == 0:
                wgut = wgu0
            else:
                wgut = pools["w"].tile([128, NKT, 256], bf16, tag="wgu", name="wgu_sb")
                nc.sync.dma_start(out=wgut, in_=wgu_dram[i])
            ht = pools["ht"].tile([128, max_tc], bf16, tag=f"ht{i}",
                                  name=f"ht{i}")
            hts.append(ht)
            for (s0, ns) in _subtiles(tc, 512):
                pg = pools["ps1"].tile([128, 512], f32, tag="pg", name="pg",
                                       bufs=3)
                pu = pools["ps1"].tile([128, 512], f32, tag="pu", name="pu",
                                       bufs=2)
                for k in range(NKT):
                    nc.tensor.matmul(pg[:, :ns], wgut[:, k, 0:128],
                                     xt[:, k, s0:s0 + ns],
                                     start=(k == 0), stop=(k == NKT - 1))
                for k in range(NKT):
                    nc.tensor.matmul(pu[:, :ns], wgut[:, k, 128:256],
                                     xt[:, k, s0:s0 + ns],
                                     start=(k == 0), stop=(k == NKT - 1))
                sg = pools["tmp"].tile([128, 512], f32, tag="sg", name="sg")
                nc.scalar.activation(sg[:, :ns], pg[:, :ns], Silu)
                nc.vector.tensor_mul(ht[:, s0:s0 + ns], sg[:, :ns], pu[:, :ns])

        if ci == 0:
            # wd / scales are first needed by stage 2 of chunk 0 — same queue
            # as the stage-1 streams, behind chunk 0's weights (always-ready
            # DMAs keep program order within one engine queue, so these can't
            # be hoisted ahead of the startup-critical bytes).
            s_sb = pools["const"].tile([128, T // 128], f32, tag="s", name="s_sb")
            nc.sync.dma_start(out=s_sb, in_=s_dram)
            for i in range(NIT):
                wdt = pools["wd"].tile([128, H], bf16, tag=f"wd{i}",
                                       name=f"wd_sb{i}")
                nc.sync.dma_start(out=wdt, in_=wd_dram[i])
                wd_tiles.append(wdt)

        for t128 in range(tc // 128):
            gt = t0 // 128 + t128
            ysb = pools["y"].tile([128, H], f32, tag="y", name="ysb")
            for h4 in range(H // 512):
                py = pools["ps2"].tile([128, 512], f32, tag="py", name="py")
                for i in range(NIT):
                    nc.tensor.matmul(py,
                                     hts[i][:, t128 * 128:(t128 + 1) * 128],
                                     wd_tiles[i][:, h4 * 512:(h4 + 1) * 512],
                                     start=(i == 0), stop=(i == NIT - 1))
                nc.scalar.activation(ysb[:, h4 * 512:(h4 + 1) * 512], py, Copy,
                                     scale=s_sb[:, gt:gt + 1])
            nc.scalar.dma_start(
                out=y_dram[t0 + t128 * 128:t0 + (t128 + 1) * 128, :], in_=ysb)


def _build_program(CA, CB):
    bf16 = mybir.dt.bfloat16
    f32 = mybir.dt.float32
    nc = bacc.Bacc("TRN2", target_bir_lowering=False, debug=False,
                   enable_asserts=False, num_devices=8)

    xat = nc.dram_tensor("xat", [NKT, 128, CA], bf16, kind="ExternalInput").ap()
    xbt = nc.dram_tensor("xbt", [NKT, 128, CB], bf16, kind="ExternalInput").ap()
    wgua = nc.dram_tensor("wgua", [NIT, 128, NKT, 256], bf16, kind="ExternalInput").ap()
    wda = nc.dram_tensor("wda", [NIT, 128, H], bf16, kind="ExternalInput").ap()
    wgub = nc.dram_tensor("wgub", [NIT, 128, NKT, 256], bf16, kind="ExternalInput").ap()
    wdb = nc.dram_tensor("wdb", [NIT, 128, H], bf16, kind="ExternalInput").ap()
    sa = nc.dram_tensor("sa", [128, CA // 128], f32, kind="ExternalInput").ap()
    sb = nc.dram_tensor("sb", [128, CB // 128], f32, kind="ExternalInput").ap()
    ya = nc.dram_tensor("ya", [CA, H], f32, kind="ExternalOutput").ap()
    yb = nc.dram_tensor("yb", [CB, H], f32, kind="ExternalOutput").ap()

    with tile.TileContext(nc) as tc:
        with tc.tile_pool(name="const", bufs=2) as p_const, \
             tc.tile_pool(name="wdp", bufs=1) as p_wd, \
             tc.tile_pool(name="xp", bufs=2) as p_x, \
             tc.tile_pool(name="wp", bufs=3) as p_w, \
             tc.tile_pool(name="htp", bufs=2) as p_ht, \
             tc.tile_pool(name="yp", bufs=2) as p_y, \
             tc.tile_pool(name="tmpp", bufs=3) as p_tmp, \
             tc.tile_pool(name="ps1", bufs=2, space="PSUM") as p_ps1, \
             tc.tile_pool(name="ps2", bufs=3, space="PSUM") as p_ps2:
            pools = {"const": p_const, "wd": p_wd, "x": p_x, "w": p_w,
                     "ht": p_ht, "y": p_y, "tmp": p_tmp,
                     "ps1": p_ps1, "ps2": p_ps2}
            _emit_phase(nc, pools, xat.rearrange("k p t -> p k t"),
                        wgua, wda, sa, ya, CA)
            _emit_phase(nc, pools, xbt.rearrange("k p t -> p k t"),
                        wgub, wdb, sb, yb, CB)

    nc.compile()
    return nc


def _pack_gate_up(wg, wu):
    # [H, I] f32 x2 -> [NIT, 128, NKT, 256] bf16:
    # [i, p, k, 0:128] = wg[k*128+p, i*128+c], [i, p, k, 128:256] = wu[...]
    tg = wg.astype(BF16).reshape(NKT, 128, NIT, 128).transpose(2, 1, 0, 3)
    tu = wu.astype(BF16).reshape(NKT, 128, NIT, 128).transpose(2, 1, 0, 3)
    return np.ascontiguousarray(np.concatenate([tg, tu], axis=3))


def _pack_down(w):
    # [I, H] f32 -> [NIT, 128, H] bf16
    return np.ascontiguousarray(w.astype(BF16).reshape(NIT, 128, H))


def _pack_tokens(x_rows, cap):
    # [n, H] f32 -> [NKT, 128, cap] bf16 (transposed, zero-padded)
    n = x_rows.shape[0]
    xt = np.zeros((H, cap), dtype=BF16)
    xt[:, :n] = x_rows.astype(BF16).T
    return np.ascontiguousarray(xt).reshape(NKT, 128, cap)


def _pack_scales(s, cap):
    # [n] f32 -> [128, cap//128] f32 where [p, j] = s[j*128+p]
    full = np.zeros(cap, dtype=F32)
    full[:s.shape[0]] = s
    return np.ascontiguousarray(full.reshape(cap // 128, 128).T)


def kernel(x, ws_gate, ws_up, ws_down, wr_gate, wr_up, wr_down,
           w_router, routing_bias, top_k):
    global LAST_PERF
    x = np.asarray(x, dtype=F32)
    ws_gate = np.asarray(ws_gate, dtype=F32)
    ws_up = np.asarray(ws_up, dtype=F32)
    ws_down = np.asarray(ws_down, dtype=F32)
    wr_gate = np.asarray(wr_gate, dtype=F32)
    wr_up = np.asarray(wr_up, dtype=F32)
    wr_down = np.asarray(wr_down, dtype=F32)
    w_router = np.asarray(w_router, dtype=F32)
    routing_bias = np.asarray(routing_bias, dtype=F32)
    k = int(top_k)

    Bv, Sv, Hv = x.shape
    nt = Bv * Sv
    x_flat = x.reshape(nt, Hv)

    idx, scores = _route(x, w_router, routing_bias, k)
    idx = idx.reshape(nt, k)
    scores = scores.reshape(nt, k).astype(F32)

    # token lists per routed expert
    tok_lists = []
    cw_lists = []
    for e in range(E):
        mask = (idx == e)
        rows = np.nonzero(mask.any(axis=1))[0]
        w = (scores * mask).sum(axis=1)[rows]
        tok_lists.append(rows)
        cw_lists.append(w.astype(F32))

    max_n = max(1, max(len(t) for t in tok_lists))
    CA = -(-max_n // 128) * 128
    rem = nt - CA
    CB = max(128, -(-rem // (8 * 128)) * 128) if rem > 0 else 128

    # shared-token slices: core 7's batch A covers [0, CA); core i's batch B
    # covers [CA + i*CB, CA + (i+1)*CB) clipped to nt
    shared_a = (0, min(CA, nt))
    shared_b = []
    for i in range(8):
        lo = min(CA + i * CB, nt)
        hi = min(CA + (i + 1) * CB, nt)
        shared_b.append((lo, hi))

    # per-core input maps
    packed_shared = (_pack_gate_up(ws_gate, ws_up), _pack_down(ws_down))
    in_maps = []
    for c in range(8):
        if c < E:
            tok = tok_lists[c]
            cw = cw_lists[c]
            wgu_a = _pack_gate_up(wr_gate[c], wr_up[c])
            wd_a = _pack_down(wr_down[c])
        else:
            lo, hi = shared_a
            tok = np.arange(lo, hi)
            cw = np.ones(hi - lo, dtype=F32)
            wgu_a, wd_a = packed_shared
        lo, hi = shared_b[c]
        tok_b = np.arange(lo, hi)
        in_maps.append({
            "xat": _pack_tokens(x_flat[tok], CA),
            "xbt": _pack_tokens(x_flat[tok_b], CB),
            "wgua": wgu_a, "wda": wd_a,
            "wgub": packed_shared[0],
            "wdb": packed_shared[1],
            "sa": _pack_scales(cw, CA),
            "sb": _pack_scales(np.ones(hi - lo, dtype=F32), CB),
        })

    key = (CA, CB)
    if key not in _NC_CACHE:
        _NC_CACHE[key] = _build_program(CA, CB)
    nc = _NC_CACHE[key]

    res = bass_utils.run_bass_kernel_spmd(nc, in_maps, core_ids=list(range(8)),
                                          trace=TRACE)
    LAST_PERF = res

    out = np.zeros((nt, Hv), dtype=F32)
    for c in range(8):
        ya = res.results[c]["ya"]
        yb = res.results[c]["yb"]
        if c < E:
            tok = tok_lists[c]
            out[tok] += ya[:len(tok)]
        else:
            lo, hi = shared_a
            out[lo:hi] += ya[:hi - lo]
        lo, hi = shared_b[c]
        if hi > lo:
            out[lo:hi] += yb[:hi - lo]
    return out.reshape(Bv, Sv, Hv)
